# Initial kernel scaffold
#
"""DepAttention kernel for Trainium2 (Bass/Tile), data-parallel over batch.

score[b,i,j] = (<val[b,i],val[b,j]> + <dep[b,i,j],dep[b,j,i]>) / sqrt(D)
out = exp(score)*adj / (rowsum(exp(score)*adj) + 1e-10)

score is symmetric in (i,j) (both terms are), so per core (one batch
element) we compute only the upper block-triangle of the 2x2 grid of
128x128 score blocks -- (0,0), (0,1), (1,1) -- and mirror (0,1) into
(1,0) with a PE transpose. The dep term dominates traffic: each 128-row
x 64-col chunk needs A = dep[iblk, jchunk, :] (contiguous) and
B' = dep[jchunk, iblk, :] with (i,j) swapped (strided AP, 512B runs).
DVE does an in-place multiply then a segmented reduce over d.
"""

import numpy as np

import concourse.bacc as bacc
import concourse.tile as tile
import concourse.mybir as mybir
from concourse.bass_utils import run_bass_kernel_spmd

B, N, D = 8, 256, 128
TJ = 64  # columns per dep chunk
SCALE = 1.0 / np.sqrt(np.float32(D))
EPS = 1e-10
F32 = mybir.dt.float32

_NC = None


def build_nc():
    nc = bacc.Bacc("TRN2", target_bir_lowering=False, debug=False, num_devices=8)

    dep = nc.dram_tensor("dep", [N, N, D], F32, kind="ExternalInput")
    valT = nc.dram_tensor("valT", [D, N], F32, kind="ExternalInput")
    adj = nc.dram_tensor("adj", [N, N], F32, kind="ExternalInput")
    ident = nc.dram_tensor("ident", [128, 128], F32, kind="ExternalInput")
    out = nc.dram_tensor("out", [N, N], F32, kind="ExternalOutput")

    with tile.TileContext(nc) as tc:
        with (
            tc.tile_pool(name="a", bufs=2) as a_pool,
            tc.tile_pool(name="b", bufs=2) as b_pool,
            tc.tile_pool(name="persist", bufs=1) as pp,
            tc.tile_pool(name="psum", bufs=1, space="PSUM") as psp,
        ):
            # persistent tiles
            vt = pp.tile([D, N], F32, tag="vt")
            id_t = pp.tile([128, 128], F32, tag="id")
            adj_t = [pp.tile([128, N], F32, tag=f"adj{i}") for i in range(2)]
            score = [pp.tile([128, N], F32, tag=f"score{i}") for i in range(2)]
            expv = [pp.tile([128, N], F32, tag=f"expv{i}") for i in range(2)]
            den = [pp.tile([128, 1], F32, tag=f"den{i}") for i in range(2)]
            rec = [pp.tile([128, 1], F32, tag=f"rec{i}") for i in range(2)]

            nc.sync.dma_start(vt[:], valT[:])
            nc.sync.dma_start(id_t[:], ident[:])
            for i in range(2):
                nc.sync.dma_start(adj_t[i][:], adj[128 * i : 128 * (i + 1), :])

            # val part: score_val[I] = valT[:, I*128:+128].T @ valT  -> PSUM
            psum_sv = [psp.tile([128, N], F32, tag=f"sv{i}") for i in range(2)]
            for i in range(2):
                nc.tensor.matmul(
                    psum_sv[i][:],
                    vt[:, 128 * i : 128 * (i + 1)],
                    vt[:],
                    start=True,
                    stop=True,
                )

            # dep part: blocks (I,J) with J >= I, chunks of TJ columns
            for (bi, bj) in ((0, 0), (0, 1), (1, 1)):
                i0 = 128 * bi
                for c in range(128 // TJ):
                    j0 = 128 * bj + c * TJ
                    a_t = a_pool.tile([128, TJ, D], F32)
                    b_t = b_pool.tile([128, TJ, D], F32)
                    nc.sync.dma_start(a_t[:], dep[i0 : i0 + 128, j0 : j0 + TJ, :])
                    nc.scalar.dma_start(
                        b_t[:], dep[j0 : j0 + TJ, i0 : i0 + 128, :].transpose([1, 0, 2])
                    )
                    nc.vector.tensor_mul(a_t[:], a_t[:], b_t[:])
                    nc.vector.reduce_sum(
                        score[bi][:, j0 : j0 + TJ], a_t[:], axis=mybir.AxisListType.X
                    )

            # mirror dep block (0,1) -> (1,0): PE transpose (reads the pure
            # dep part of score0 before val is added in-place below)
            psum_t = psp.tile([128, 128], F32, tag="pt")
            nc.tensor.transpose(psum_t[:], score[0][:, 128:256], id_t[:])
            nc.scalar.copy(score[1][:, 0:128], psum_t[:])

            # epilogue per row-block
            for i in range(2):
                nc.vector.tensor_add(score[i][:], score[i][:], psum_sv[i][:])
                nc.scalar.activation(
                    expv[i][:],
                    score[i][:],
                    mybir.ActivationFunctionType.Exp,
                    scale=float(SCALE),
                )
                nc.vector.tensor_mul(expv[i][:], expv[i][:], adj_t[i][:])
                nc.vector.reduce_sum(den[i][:], expv[i][:], axis=mybir.AxisListType.X)
                nc.vector.tensor_scalar_add(den[i][:], den[i][:], float(EPS))
                nc.vector.reciprocal(rec[i][:], den[i][:])
                nc.vector.tensor_scalar_mul(expv[i][:], expv[i][:], rec[i][:, 0:1])
                nc.sync.dma_start(out[128 * i : 128 * (i + 1), :], expv[i][:])

    nc.compile()
    return nc


def _get_nc():
    global _NC
    if _NC is None:
        _NC = build_nc()
    return _NC


def kernel(val_out, dep_embed, adj):
    val_out = np.asarray(val_out, dtype=np.float32)
    dep_embed = np.asarray(dep_embed, dtype=np.float32)
    adj = np.asarray(adj, dtype=np.float32)
    assert val_out.shape == (B, N, D)
    assert dep_embed.shape == (B, N, N, D)
    assert adj.shape == (B, N, N)

    nc = _get_nc()
    ident = np.eye(128, dtype=np.float32)
    in_maps = [
        {
            "dep": np.ascontiguousarray(dep_embed[b]),
            "valT": np.ascontiguousarray(val_out[b].T),
            "adj": np.ascontiguousarray(adj[b]),
            "ident": ident,
        }
        for b in range(B)
    ]
    res = run_bass_kernel_spmd(nc, in_maps, core_ids=list(range(B)))
    return np.stack([r["out"] for r in res.results])


# revision 5
# speedup vs baseline: 1.6843x; 1.6843x over previous
"""DepAttention kernel for Trainium2 (Bass/Tile), data-parallel over batch.

score[b,i,j] = (<val[b,i],val[b,j]> + <dep[b,i,j],dep[b,j,i]>) / sqrt(D)
out = exp(score)*adj / (rowsum(exp(score)*adj) + 1e-10)

score is symmetric in (i,j) (both terms are), so per core (one batch
element) we compute only the upper block-triangle of the 2x2 grid of
128x128 score blocks -- (0,0), (0,1), (1,1) -- and mirror (0,1) into
(1,0) with a PE transpose. The dep term dominates traffic: each 128-row
x 64-col chunk needs A = dep[iblk, jchunk, :] (contiguous) and
B' = dep[jchunk, iblk, :] with (i,j) swapped (strided AP, 512B runs).
DVE does an in-place multiply then a segmented reduce over d.
"""

import numpy as np

import concourse.bacc as bacc
import concourse.tile as tile
import concourse.mybir as mybir
from concourse.bass_utils import run_bass_kernel_spmd

B, N, D = 8, 256, 128
TJ = 64  # columns per dep chunk
SCALE = 1.0 / np.sqrt(np.float32(D))
EPS = 1e-10
F32 = mybir.dt.float32

_NC = None


def build_nc(reps=1):
    """reps>1 unrolls the whole computation N times (for timing: the
    wall-clock delta between reps=R and reps=1 isolates device time)."""
    nc = bacc.Bacc("TRN2", target_bir_lowering=False, debug=False, num_devices=8)

    dep = nc.dram_tensor("dep", [N, N, D], F32, kind="ExternalInput")
    valT = nc.dram_tensor("valT", [D, N], F32, kind="ExternalInput")
    adj = nc.dram_tensor("adj", [N, N], F32, kind="ExternalInput")
    ident = nc.dram_tensor("ident", [128, 128], F32, kind="ExternalInput")
    out = nc.dram_tensor("out", [N, N], F32, kind="ExternalOutput")

    with tile.TileContext(nc) as tc:
        with (
            tc.tile_pool(name="a", bufs=2) as a_pool,
            tc.tile_pool(name="b", bufs=2) as b_pool,
            tc.tile_pool(name="persist", bufs=1) as pp,
            tc.tile_pool(name="psum", bufs=1, space="PSUM") as psp,
        ):
            # persistent tiles
            vt = pp.tile([D, N], F32, tag="vt")
            id_t = pp.tile([128, 128], F32, tag="id")
            adj_t = [
                pp.tile([128, N], F32, tag=f"adj{i}", name=f"adj{i}") for i in range(2)
            ]
            score = [
                pp.tile([128, N], F32, tag=f"score{i}", name=f"score{i}")
                for i in range(2)
            ]
            expv = [
                pp.tile([128, N], F32, tag=f"expv{i}", name=f"expv{i}")
                for i in range(2)
            ]
            den = [
                pp.tile([128, 1], F32, tag=f"den{i}", name=f"den{i}") for i in range(2)
            ]
            rec = [
                pp.tile([128, 1], F32, tag=f"rec{i}", name=f"rec{i}") for i in range(2)
            ]

            nc.sync.dma_start(vt[:], valT[:])
            nc.sync.dma_start(id_t[:], ident[:])
            for i in range(2):
                nc.sync.dma_start(adj_t[i][:], adj[128 * i : 128 * (i + 1), :])

            psum_sv = [
                psp.tile([128, N], F32, tag=f"sv{i}", name=f"sv{i}") for i in range(2)
            ]

            for _rep in range(reps):
                # val part: score_val[I] = valT[:, I*128:+128].T @ valT -> PSUM
                for i in range(2):
                    nc.tensor.matmul(
                        psum_sv[i][:],
                        vt[:, 128 * i : 128 * (i + 1)],
                        vt[:],
                        start=True,
                        stop=True,
                    )

                # dep part: blocks (I,J) with J >= I, chunks of TJ columns
                for (bi, bj) in ((0, 0), (0, 1), (1, 1)):
                    i0 = 128 * bi
                    for c in range(128 // TJ):
                        j0 = 128 * bj + c * TJ
                        a_t = a_pool.tile([128, TJ, D], F32, name="a_t")
                        b_t = b_pool.tile([128, TJ, D], F32, name="b_t")
                        nc.sync.dma_start(a_t[:], dep[i0 : i0 + 128, j0 : j0 + TJ, :])
                        nc.scalar.dma_start(
                            b_t[:],
                            dep[j0 : j0 + TJ, i0 : i0 + 128, :].transpose([1, 0, 2]),
                        )
                        nc.vector.tensor_mul(a_t[:], a_t[:], b_t[:])
                        nc.vector.reduce_sum(
                            score[bi][:, j0 : j0 + TJ],
                            a_t[:],
                            axis=mybir.AxisListType.X,
                        )

                # mirror dep block (0,1) -> (1,0): PE transpose (reads the
                # pure dep part of score0 before val is added in-place below)
                psum_t = psp.tile([128, 128], F32, tag="pt", name="pt")
                nc.tensor.transpose(psum_t[:], score[0][:, 128:256], id_t[:])
                nc.scalar.copy(score[1][:, 0:128], psum_t[:])

                # epilogue per row-block
                for i in range(2):
                    nc.vector.tensor_add(score[i][:], score[i][:], psum_sv[i][:])
                    nc.scalar.activation(
                        expv[i][:],
                        score[i][:],
                        mybir.ActivationFunctionType.Exp,
                        scale=float(SCALE),
                    )
                    nc.vector.tensor_mul(expv[i][:], expv[i][:], adj_t[i][:])
                    nc.vector.reduce_sum(
                        den[i][:], expv[i][:], axis=mybir.AxisListType.X
                    )
                    nc.vector.tensor_scalar_add(den[i][:], den[i][:], float(EPS))
                    nc.vector.reciprocal(rec[i][:], den[i][:])
                    nc.vector.tensor_scalar_mul(expv[i][:], expv[i][:], rec[i][:, 0:1])
                    nc.sync.dma_start(out[128 * i : 128 * (i + 1), :], expv[i][:])

    nc.compile()
    return nc


def _get_nc():
    global _NC
    if _NC is None:
        _NC = build_nc()
    return _NC


def kernel(val_out, dep_embed, adj):
    val_out = np.asarray(val_out, dtype=np.float32)
    dep_embed = np.asarray(dep_embed, dtype=np.float32)
    adj = np.asarray(adj, dtype=np.float32)
    assert val_out.shape == (B, N, D)
    assert dep_embed.shape == (B, N, N, D)
    assert adj.shape == (B, N, N)

    nc = _get_nc()
    ident = np.eye(128, dtype=np.float32)
    in_maps = [
        {
            "dep": np.ascontiguousarray(dep_embed[b]),
            "valT": np.ascontiguousarray(val_out[b].T),
            "adj": np.ascontiguousarray(adj[b]),
            "ident": ident,
        }
        for b in range(B)
    ]
    res = run_bass_kernel_spmd(nc, in_maps, core_ids=list(range(B)))
    return np.stack([r["out"] for r in res.results])
